# revision 17
# baseline (speedup 1.0000x reference)
"""Trainium2 Bass kernel for a 2-layer GCN + linear head + log_softmax
(nn_Detector_57604101373957).

Strategy (8 NeuronCores, SPMD, feature-major aggregation on the PE array):
  - Nodes are dealt to cores by global degree rank, each core's 12.5k nodes
    degree-sorted into 98 tiles of 128 slots (as in the earlier round-robin
    design).  Edges are partitioned by destination; the host pre-gathers
    per-edge messages (with the full dis[src]*dis[dst] norm folded in) into
    fp8e4m3 slabs laid out FEATURE-major: partition dim = 2x64 features of a
    tile PAIR, free dim = 128 destination slots.  One slab block per
    (group, round).
  - Aggregation = identity-matmul accumulation into PSUM on the tensor
    engine (2.4 GHz, fp8 moving operand; DoubleRow sums two rounds per
    instruction).  This keeps the DVE free and makes fp8 streaming usable
    (DVE cannot pack fp8, PE can).
  - Feature-major means zero transposes: h1T = Relu(psum + b1) directly via
    the scalar engine (bias is per-partition), layer-2 contribution
    s2T = W2_blockdiag^T @ h1T with a stationary weight, and the 2-class
    head z = w3pair^T @ h2T as a [2, N] matmul.
  - Three dispatches: X computes yT = W1^T @ (dis*x)T per core (so layer-1
    messages are 64-wide, halving slab bytes); host all-gathers y and builds
    slab1; A aggregates layer 1 + applies W2; host gathers slab2 from the
    shard outputs; B aggregates layer 2 + head + log-softmax.
"""
import os
import sys
import time

sys.path.insert(0, '/opt/trn_rl_repo')

# This kernel needs the axon-tunneled NeuronCores; undo a cpu-only pin if jax
# hasn't been initialized yet.
_jp = os.environ.get("JAX_PLATFORMS")
if _jp and "axon" not in _jp and "jax" not in sys.modules:
    os.environ.pop("JAX_PLATFORMS", None)

import numpy as np
import ml_dtypes

NCORES = 8
HID = 64
P = 128
GRP = 4           # tile-pairs per PSUM bank group (4*128 slots = 512 f32)
CHUNK_B = 4096    # slab DMA chunk size per partition (bytes)
USE_DR = bool(int(os.environ.get("GCN_USE_DOUBLEROW", "1")))
SLAB_SCALE = 64.0  # pre-quantization gain keeping fp8e4m3 in its normal range

_DEBUG = bool(int(os.environ.get("GCN_KERNEL_DEBUG", "0")))


def _log(*a):
    if _DEBUG:
        print("[kernel]", *a, flush=True)


# ----------------------------------------------------------------------------
# toolchain workarounds
# ----------------------------------------------------------------------------
_patched = False


def _apply_patches():
    """This walrus build accepts only ONE semaphore wait per instruction.
    Split Tile's tail-drain waits and any multi-wait instruction onto NOPs."""
    global _patched
    if _patched:
        return
    _patched = True
    import concourse.tile as tile_mod
    from bass_rust import ScopedClock

    def _drain_and_barrier(self, tick_clock, wait_clock):
        nc = self.nc
        import concourse.mybir as mybir
        sink = nc.sync.nop(nofuse=True)
        sink_inst = sink.ins if hasattr(sink, "ins") else sink
        wait_clock.add_sem_waits(sink_inst,
                                ScopedClock({None: tick_clock.global_clock}))
        si = sink_inst.sync_info
        waits = list(si.on_wait) if si is not None and si.on_wait else []
        if len(waits) > 1:
            si.on_wait = waits[:1]
            for k, w in enumerate(waits[1:]):
                extra = nc.sync.nop(nofuse=True)
                extra_inst = extra.ins if hasattr(extra, "ins") else extra
                esi = extra_inst.sync_info
                if esi is None:
                    extra_inst.sync_info = mybir.SyncInfo(on_wait=[w],
                                                          on_update=[])
                else:
                    esi.on_wait = [w]
        nc.sync.drain()
        nc.all_engine_barrier()
        assert self.sems is not None
        popped = nc._tile_sem_poison_stack.pop()
        assert popped is self._sem_poison
        nc.clear_and_free_semaphores(list(self.sems.allocated().values()))
        nc.all_engine_barrier()

    tile_mod.TileContext._drain_and_barrier = _drain_and_barrier


def _split_multi_waits(nc):
    import concourse.mybir as mybir
    n = 0
    for fn in nc.m.functions:
        for bb in fn.blocks:
            new_insts = []
            for inst in bb.instructions:
                si = inst.sync_info
                if si is not None and si.on_wait and len(si.on_wait) > 1:
                    waits = list(si.on_wait)
                    for k, w in enumerate(waits[:-1]):
                        nop = mybir.InstNoOp(
                            name=f"{inst.name}-wsplit{k}",
                            sync_info=mybir.SyncInfo(on_wait=[w],
                                                     on_update=[]),
                            bass_nofuse=True,
                            engine=inst.engine,
                        )
                        new_insts.append(nop)
                    si.on_wait = waits[-1:]
                    n += 1
                new_insts.append(inst)
            bb.instructions[:] = new_insts
    return n


# ----------------------------------------------------------------------------
# SPMD runner (compile once, run; mirrors bass2jax.run_bass_via_pjrt)
# ----------------------------------------------------------------------------
class _Runner:
    def __init__(self, nc, n_cores=NCORES, replicated=()):
        import jax
        from jax.sharding import Mesh, PartitionSpec
        from jax.experimental.shard_map import shard_map
        import concourse.mybir as mybir
        from concourse.bass2jax import (_bass_exec_p, install_neuronx_cc_hook,
                                        partition_id_tensor)
        install_neuronx_cc_hook()
        _split_multi_waits(nc)
        self.jax = jax
        self.n_cores = n_cores
        in_names, out_names, out_avals, zero_outs = [], [], [], []
        pname = nc.partition_id_tensor.name if nc.partition_id_tensor else None
        for alloc in nc.m.functions[0].allocations:
            if not isinstance(alloc, mybir.MemoryLocationSet):
                continue
            name = alloc.memorylocations[0].name
            if alloc.kind == "ExternalInput":
                if name != pname:
                    in_names.append(name)
            elif alloc.kind == "ExternalOutput":
                out_names.append(name)
                shape = tuple(alloc.tensor_shape)
                dtype = mybir.dt.np(alloc.dtype)
                out_avals.append(jax.core.ShapedArray(shape, dtype))
                zero_outs.append(np.zeros(shape, dtype))
        self.in_names, self.out_names = in_names, out_names
        self.out_avals, self.zero_outs = out_avals, zero_outs
        all_in = in_names + out_names + ([pname] if pname else [])

        def _body(*args):
            operands = list(args)
            if pname is not None:
                operands.append(partition_id_tensor())
            return tuple(_bass_exec_p.bind(
                *operands,
                out_avals=tuple(out_avals),
                in_names=tuple(all_in),
                out_names=tuple(out_names),
                lowering_input_output_aliases=(),
                sim_require_finite=True,
                sim_require_nnan=True,
                nc=nc,
            ))

        devices = jax.devices()[:n_cores]
        self.mesh = Mesh(np.asarray(devices), ("core",))
        self.replicated = set(replicated)
        in_specs = tuple(
            PartitionSpec() if name in self.replicated else PartitionSpec("core")
            for name in in_names) + (PartitionSpec("core"),) * len(out_names)
        out_specs = (PartitionSpec("core"),) * len(out_names)
        self.fn = jax.jit(shard_map(_body, mesh=self.mesh, in_specs=in_specs,
                                    out_specs=out_specs, check_rep=False),
                          keep_unused=True)
        self._staged = None

    def stage(self, in_maps):
        from jax.sharding import NamedSharding, PartitionSpec
        n = self.n_cores
        sh = NamedSharding(self.mesh, PartitionSpec("core"))
        shr = NamedSharding(self.mesh, PartitionSpec())
        staged = []
        for name in self.in_names:
            if name in self.replicated:
                staged.append(self.jax.device_put(
                    np.asarray(in_maps[0][name]), shr))
            else:
                staged.append(self.jax.device_put(np.concatenate(
                    [np.asarray(in_maps[c][name]) for c in range(n)], axis=0),
                    sh))
        staged += [self.jax.device_put(
            np.zeros((n * z.shape[0], *z.shape[1:]), z.dtype), sh)
            for z in self.zero_outs]
        self._staged = staged
        self.jax.block_until_ready(self._staged)

    def run(self):
        out = self.fn(*self._staged)
        self.jax.block_until_ready(out)
        n = self.n_cores
        out = [np.asarray(o) for o in out]
        return [{name: out[i].reshape(n, *self.out_avals[i].shape)[c]
                 for i, name in enumerate(self.out_names)}
                for c in range(n)]

    def time_once(self):
        t0 = time.perf_counter()
        out = self.fn(*self._staged)
        self.jax.block_until_ready(out)
        return time.perf_counter() - t0

    def time_pipelined(self, n_iter=10, warmup=2):
        for _ in range(warmup):
            out = self.fn(*self._staged)
        self.jax.block_until_ready(out)
        t0 = time.perf_counter()
        outs = [self.fn(*self._staged) for _ in range(n_iter)]
        self.jax.block_until_ready(outs)
        return (time.perf_counter() - t0) / n_iter


# ----------------------------------------------------------------------------
# host-side graph preparation
# ----------------------------------------------------------------------------
def _prep_graph(edge_index, n):
    src = np.asarray(edge_index[0], dtype=np.int64)
    dst = np.asarray(edge_index[1], dtype=np.int64)
    deg = np.bincount(dst, minlength=n).astype(np.int64) + 1  # + self loop

    # deal nodes to cores by global degree rank
    order = np.argsort(-deg, kind="stable")          # rank -> orig node
    rank_of = np.empty(n, dtype=np.int64)
    rank_of[order] = np.arange(n)
    core_of = rank_of % NCORES
    slot_of = rank_of // NCORES                      # degree-sorted per core

    per_core = (n + NCORES - 1) // NCORES
    tiles = (per_core + P - 1) // P
    slots = tiles * P
    newid = core_of * slots + slot_of                # orig -> new id

    # common per-tile round counts: max degree of slot t*128 across cores
    # (slots are degree-sorted descending, so tile max = first slot's degree)
    D = np.zeros(tiles, dtype=np.int64)
    deg_sorted = deg[order]                          # descending
    for t in range(tiles):
        s0 = t * P
        ranks = s0 * NCORES + np.arange(NCORES)      # first slot of tile t
        ranks = ranks[ranks < n]
        D[t] = deg_sorted[ranks].max() if len(ranks) else 1
    R = int(D.sum())
    offs = np.zeros(tiles, dtype=np.int64)
    offs[1:] = np.cumsum(D)[:-1]

    # in-edge lists grouped by destination (new-id space), self-loops first
    e_order = np.argsort(newid[dst], kind="stable")
    sdst_new = newid[dst][e_order]
    ssrc_new = newid[src][e_order]
    starts = np.searchsorted(sdst_new, np.arange(NCORES * slots), side="left")
    ends = np.searchsorted(sdst_new, np.arange(NCORES * slots), side="right")

    # per-core gather index arrays [128, R] (new-id space), -1 for pad
    nid_grid = np.full((NCORES, slots), -1, dtype=np.int64)
    nid_grid[core_of[order], slot_of[order]] = order  # orig ids on the grid
    idx_new = np.full((NCORES, P, R), -1, dtype=np.int64)
    for t in range(tiles):
        dt = int(D[t])
        o = int(offs[t])
        for c in range(NCORES):
            base = c * slots + t * P
            for p in range(P):
                v = nid_grid[c, t * P + p]
                if v < 0:
                    continue
                nv = base + p
                s, e = starts[nv], ends[nv]
                cnt = e - s
                # self-loop first, then in-edges
                idx_new[c, p, o] = nv
                m = min(cnt, dt - 1)
                idx_new[c, p, o + 1:o + 1 + m] = ssrc_new[s:s + m]
                assert cnt <= dt - 1, (cnt, dt)
    return dict(order=order, newid=newid, core_of=core_of, slot_of=slot_of,
                deg=deg, tiles=tiles, slots=slots, D=D, R=R, offs=offs,
                idx_new=idx_new, nid_grid=nid_grid)


def _pair_groups(tiles, D):
    """Uniform-width group/round structure shared by host slab layout and the
    device programs.  Returns (npairs, groups, chunks) where groups is a list
    of (pair0, wg, Rg) and chunks[g] is a list of round counts per DMA."""
    assert tiles % 2 == 0
    npairs = tiles // 2
    Dp = [int(D[2 * j]) for j in range(npairs)]      # per-pair rounds
    # DP over group sizes 1..GRP minimizing padded slab columns sum(wg*Rg)
    # plus instruction overheads: each round-matmul ~1.5 column-equivalents
    # of issue cost, each group ~12 (epilogue ops + weight swap).
    INF = 1 << 60
    OH_ROUND, OH_GROUP = 3, 24                       # in half-columns
    best = [INF] * (npairs + 1)
    bsz = [0] * (npairs + 1)
    best[npairs] = 0
    for j in range(npairs - 1, -1, -1):
        for s in range(1, min(GRP, npairs - j) + 1):
            c = 2 * s * Dp[j] + OH_ROUND * Dp[j] + OH_GROUP + best[j + s]
            if c < best[j]:
                best[j], bsz[j] = c, s
    groups = []
    j = 0
    while j < npairs:
        wg = bsz[j]
        groups.append((j, wg, Dp[j]))                # D sorted desc -> max
        j += wg
    chunks = []
    for (j0, wg, Rg) in groups:
        rpc = max(1, CHUNK_B // (wg * P))            # rounds per chunk
        ch = []
        r = 0
        while r < Rg:
            ch.append(min(rpc, Rg - r))
            r += rpc
        chunks.append(ch)
    return npairs, Dp, groups, chunks


def _slab_cols(groups, Dp, D, offs):
    """Column index arrays (into the per-core [P, R] message grid, R = pad)
    in exact device-emission order.  Returns (cols_even, cols_odd, ncolblk)."""
    ce, co = [], []
    R = int(D.sum())
    for (j0, wg, Rg) in groups:
        for r in range(Rg):
            for jj in range(wg):
                j = j0 + jj
                te, to = 2 * j, 2 * j + 1
                ce.append(offs[te] + r if r < D[te] else R)
                co.append(offs[to] + r if r < D[to] else R)
    return np.asarray(ce, dtype=np.int64), np.asarray(co, dtype=np.int64)


def _build_slab(msgv, cols_even, cols_odd):
    """msgv: [P(slot), R+1, HID] f32 message grid (last col all-zero).
    Returns the fp8 feature-major slab bytes [128, K*128] uint8."""
    K = len(cols_even)
    top = msgv[:, cols_even, :]                      # [128, K, 64]
    bot = msgv[:, cols_odd, :]
    # -> [64 feats, K, 128 slots] stacked to [128, K*128]
    slab = np.empty((2 * HID, K, P), dtype=np.float32)
    slab[:HID] = top.transpose(2, 1, 0)
    slab[HID:] = bot.transpose(2, 1, 0)
    slab *= SLAB_SCALE
    slab8 = slab.reshape(2 * HID, K * P).astype(ml_dtypes.float8_e4m3fn)
    return slab8.view(np.uint8)


# ----------------------------------------------------------------------------
# bass programs
# ----------------------------------------------------------------------------
def _emit_agg(nc, mpool, pspool, msgd, id2f8, groups, chunks, mcol_state):
    """Emit the aggregation matmul stream for one group list; returns a list
    of per-group PSUM tiles (caller emits epilogues with 1-group lookahead)."""
    import concourse.mybir as mybir
    DR = mybir.MatmulPerfMode.DoubleRow
    f32 = mybir.dt.float32
    id3 = id2f8.rearrange("k (s m) -> k s m", s=2)

    def agg_group(g):
        (j0, wg, Rg) = groups[g]
        wcols = wg * P
        ps = pspool.tile([P, GRP * P], f32, tag="agg")
        r = 0
        for nrounds in chunks[g]:
            cb = nrounds * wcols
            chunk = mpool.tile([P, CHUNK_B], mybir.dt.uint8, tag="chunk")
            nc.sync.dma_start(out=chunk[:, :cb],
                              in_=msgd[:, mcol_state[0]:mcol_state[0] + cb])
            mcol_state[0] += cb
            cf8 = chunk[:].bitcast(mybir.dt.float8e4)
            roff = 0
            rleft = nrounds
            while rleft:
                if USE_DR and rleft >= 2:
                    rhs = cf8[:, roff:roff + 2 * wcols].rearrange(
                        "k (s n) -> k s n", s=2)
                    nc.tensor.matmul(out=ps[:, :wcols], lhsT=id3, rhs=rhs,
                                     start=(r == 0), stop=(r + 2 == Rg),
                                     perf_mode=DR)
                    r += 2
                    roff += 2 * wcols
                    rleft -= 2
                else:
                    nc.tensor.matmul(out=ps[:, :wcols], lhsT=id2f8[:, :P],
                                     rhs=cf8[:, roff:roff + wcols],
                                     start=(r == 0), stop=(r + 1 == Rg))
                    r += 1
                    roff += wcols
                    rleft -= 1
        return ps

    return agg_group


def _build_A(groups, chunks, M, npairs):
    """Layer-1 aggregation (of host-gathered y messages) + W2 transform.
    Output shard2 f16 [128, npairs*128]: partition q<64 = feat q of even
    tile, q>=64 = feat q-64 of odd tile; col = pair*128 + slot."""
    import concourse.bass as bass
    import concourse.mybir as mybir
    import concourse.tile as tile
    f32, f16, bf16 = mybir.dt.float32, mybir.dt.float16, mybir.dt.bfloat16
    u8 = mybir.dt.uint8

    nc = bass.Bass()
    msgd = nc.dram_tensor("msgd", [P, M], u8, kind="ExternalInput")
    id2d = nc.dram_tensor("id2d", [P, 2 * P], u8, kind="ExternalInput")
    w2blkd = nc.dram_tensor("w2blkd", [P, P], bf16, kind="ExternalInput")
    b1paird = nc.dram_tensor("b1paird", [P, 1], f32, kind="ExternalInput")
    shard2 = nc.dram_tensor("shard2", [P, npairs * P], f16,
                            kind="ExternalOutput")

    with tile.TileContext(nc) as tc:
        with tc.tile_pool(name="const", bufs=1) as cpool, \
             tc.tile_pool(name="msg", bufs=6) as mpool, \
             tc.tile_pool(name="ep", bufs=2) as epool, \
             tc.tile_pool(name="ps", bufs=2, space="PSUM") as pspool:
            id2 = cpool.tile([P, 2 * P], u8)
            nc.sync.dma_start(out=id2[:], in_=id2d[:, :])
            w2blk = cpool.tile([P, P], bf16)
            nc.sync.dma_start(out=w2blk[:], in_=w2blkd[:, :])
            b1p = cpool.tile([P, 1], f32)
            nc.sync.dma_start(out=b1p[:], in_=b1paird[:, :])
            id2f8 = id2[:].bitcast(mybir.dt.float8e4)

            mcol = [0]
            agg_group = _emit_agg(nc, mpool, pspool, msgd, id2f8, groups,
                                  chunks, mcol)

            def epilogue(g, ps):
                (j0, wg, Rg) = groups[g]
                wcols = wg * P
                h1 = epool.tile([P, GRP * P], bf16, tag="h1")
                nc.scalar.activation(out=h1[:, :wcols], in_=ps[:, :wcols],
                                     func=mybir.ActivationFunctionType.Relu,
                                     bias=b1p[:], scale=1.0 / SLAB_SCALE)
                ps2 = pspool.tile([P, GRP * P], f32, tag="ps2")
                nc.tensor.matmul(out=ps2[:, :wcols], lhsT=w2blk[:],
                                 rhs=h1[:, :wcols], start=True, stop=True)
                s2 = epool.tile([P, GRP * P], f16, tag="s2")
                nc.vector.tensor_copy(out=s2[:, :wcols], in_=ps2[:, :wcols])
                nc.sync.dma_start(
                    out=shard2[:, j0 * P:j0 * P + wcols], in_=s2[:, :wcols])

            prev = None
            for g in range(len(groups)):
                ps = agg_group(g)
                if prev is not None:
                    epilogue(g - 1, prev)
                prev = ps
            epilogue(len(groups) - 1, prev)
            assert mcol[0] == M, (mcol[0], M)
    return nc


def _build_B(groups, chunks, M, npairs, b3diff):
    """Layer-2 aggregation + relu + 2-class head + log_softmax.
    z' rows [2, N] are bounced through a DRAM scratch tile into a dense
    [128, 98] layout so the head math runs on full partitions.
    Output outd f32 [128, 2*tiles]: out[p, 2*(jj*2+c)/..]: col q<tiles is
    lp0 for new slot (q)*128+p remapped as q=jj*2+c; col tiles+q is lp1."""
    import concourse.bass as bass
    import concourse.mybir as mybir
    import concourse.tile as tile
    f32, f16, bf16 = mybir.dt.float32, mybir.dt.float16, mybir.dt.bfloat16
    u8 = mybir.dt.uint8
    N = npairs * P
    tiles = 2 * npairs

    nc = bass.Bass()
    msgd = nc.dram_tensor("msg2d", [P, M], u8, kind="ExternalInput")
    id2d = nc.dram_tensor("id2d", [P, 2 * P], u8, kind="ExternalInput")
    w3paird = nc.dram_tensor("w3paird", [P, 2], bf16, kind="ExternalInput")
    b2paird = nc.dram_tensor("b2paird", [P, 1], f32, kind="ExternalInput")
    outd = nc.dram_tensor("out", [P, 2 * tiles], f32, kind="ExternalOutput")

    with tile.TileContext(nc) as tc:
        with tc.tile_pool(name="const", bufs=1) as cpool, \
             tc.tile_pool(name="msg", bufs=6) as mpool, \
             tc.tile_pool(name="ep", bufs=2) as epool, \
             tc.tile_pool(name="z", bufs=1) as zpool, \
             tc.tile_pool(name="zd", bufs=1, space="DRAM") as zdpool, \
             tc.tile_pool(name="ps", bufs=2, space="PSUM") as pspool:
            id2 = cpool.tile([P, 2 * P], u8)
            nc.sync.dma_start(out=id2[:], in_=id2d[:, :])
            w3p = cpool.tile([P, 2], bf16)
            nc.sync.dma_start(out=w3p[:], in_=w3paird[:, :])
            b2p = cpool.tile([P, 1], f32)
            nc.sync.dma_start(out=b2p[:], in_=b2paird[:, :])
            id2f8 = id2[:].bitcast(mybir.dt.float8e4)

            zstage = zpool.tile([2, N], f32)

            mcol = [0]
            agg_group = _emit_agg(nc, mpool, pspool, msgd, id2f8, groups,
                                  chunks, mcol)

            def epilogue(g, ps):
                (j0, wg, Rg) = groups[g]
                wcols = wg * P
                h2 = epool.tile([P, GRP * P], bf16, tag="h2")
                nc.scalar.activation(out=h2[:, :wcols], in_=ps[:, :wcols],
                                     func=mybir.ActivationFunctionType.Relu,
                                     bias=b2p[:], scale=1.0 / SLAB_SCALE)
                psz = pspool.tile([2, GRP * P], f32, tag="psz")
                nc.tensor.matmul(out=psz[:, :wcols], lhsT=w3p[:],
                                 rhs=h2[:, :wcols], start=True, stop=True)
                # z' = z + b3diff staged into the dense z row-pair
                nc.vector.tensor_scalar_add(
                    out=zstage[:, j0 * P:j0 * P + wcols], in0=psz[:, :wcols],
                    scalar1=float(b3diff))

            prev = None
            for g in range(len(groups)):
                ps = agg_group(g)
                if prev is not None:
                    epilogue(g - 1, prev)
                prev = ps
            epilogue(len(groups) - 1, prev)
            assert mcol[0] == M, (mcol[0], M)

            # bounce z' [2, N] -> DRAM -> dense [128, tiles] (q = jj*2 + c)
            zdram = zdpool.tile([2, N], f32)
            nc.sync.dma_start(out=zdram[:], in_=zstage[:])
            zd = zpool.tile([P, tiles], f32)
            nc.sync.dma_start(
                out=zd[:].rearrange("p (c j) -> p c j", c=2),
                in_=zdram[:].rearrange("c (j p) -> p c j", p=P))

            # head: lp0 = -sp(z'), lp1 = z' - sp(z');
            # stable sp(z) = m + ln(1 + exp(z - 2m)), m = relu(z)
            Act = mybir.ActivationFunctionType
            m = zpool.tile([P, tiles], f32)
            nc.scalar.activation(out=m[:], in_=zd[:], func=Act.Relu)
            e = zpool.tile([P, tiles], f32)
            nc.vector.tensor_scalar(out=e[:], in0=m[:], scalar1=-2.0,
                                    scalar2=None, op0=mybir.AluOpType.mult)
            nc.vector.tensor_tensor(out=e[:], in0=e[:], in1=zd[:],
                                    op=mybir.AluOpType.add)
            nc.scalar.activation(out=e[:], in_=e[:], func=Act.Exp)
            # u = ln(e + 1)
            nc.scalar.activation(out=e[:], in_=e[:], func=Act.Ln, bias=1.0)
            sp = zpool.tile([P, tiles], f32)
            nc.vector.tensor_tensor(out=sp[:], in0=m[:], in1=e[:],
                                    op=mybir.AluOpType.add)
            lp = zpool.tile([P, 2 * tiles], f32)
            nc.vector.tensor_scalar_mul(out=lp[:, :tiles], in0=sp[:],
                                        scalar1=-1.0)
            nc.vector.tensor_tensor(out=lp[:, tiles:], in0=zd[:], in1=sp[:],
                                    op=mybir.AluOpType.subtract)
            nc.sync.dma_start(out=outd[:, :], in_=lp[:])
    return nc


def _build_X(slots):
    """yT = W1^T @ xT per core: xT f16 [128 feats, slots] (dis-scaled x),
    output yd f16 [64, slots]."""
    import concourse.bass as bass
    import concourse.mybir as mybir
    import concourse.tile as tile
    f32, f16 = mybir.dt.float32, mybir.dt.float16

    nc = bass.Bass()
    xTd = nc.dram_tensor("xTd", [P, slots], f16, kind="ExternalInput")
    w1d = nc.dram_tensor("w1d", [P, HID], f16, kind="ExternalInput")
    yd = nc.dram_tensor("yd", [HID, slots], f16, kind="ExternalOutput")

    with tile.TileContext(nc) as tc:
        with tc.tile_pool(name="const", bufs=1) as cpool, \
             tc.tile_pool(name="st", bufs=4) as spool, \
             tc.tile_pool(name="ps", bufs=4, space="PSUM") as pspool:
            w1 = cpool.tile([P, HID], f16)
            nc.sync.dma_start(out=w1[:], in_=w1d[:, :])
            xts = cpool.tile([P, slots], f16)
            nc.sync.dma_start(out=xts[:], in_=xTd[:, :])
            CN = 512
            nch = (slots + CN - 1) // CN
            for k in range(nch):
                c0 = k * CN
                w = min(CN, slots - c0)
                psy = pspool.tile([HID, CN], f32, tag="psy")
                nc.tensor.matmul(out=psy[:, :w], lhsT=w1[:],
                                 rhs=xts[:, c0:c0 + w], start=True, stop=True)
                yst = spool.tile([HID, CN], f16, tag="yst")
                if k % 2 == 0:
                    nc.vector.tensor_copy(out=yst[:, :w], in_=psy[:, :w])
                else:
                    nc.scalar.activation(
                        out=yst[:, :w], in_=psy[:, :w],
                        func=mybir.ActivationFunctionType.Copy)
                nc.sync.dma_start(out=yd[:, c0:c0 + w], in_=yst[:, :w])
    return nc


# ----------------------------------------------------------------------------
# main entry
# ----------------------------------------------------------------------------
def kernel(x, edge_index, W1, b1, W2, b2, W3, b3):
    _apply_patches()
    x = np.asarray(x, dtype=np.float32)
    n, n_feat = x.shape
    t_start = time.time()
    g = _prep_graph(edge_index, n)
    tiles, slots, D, R = g["tiles"], g["slots"], g["D"], g["R"]
    offs, idx_new, nid_grid = g["offs"], g["idx_new"], g["nid_grid"]
    newid = g["newid"]
    deg = g["deg"]
    dis = (1.0 / np.sqrt(deg)).astype(np.float32)

    npairs, Dp, groups, chunks = _pair_groups(tiles, D)
    cols_even, cols_odd = _slab_cols(groups, Dp, D, offs)
    K = len(cols_even)
    M = K * P
    _log(f"prep {time.time()-t_start:.1f}s tiles={tiles} R={R} K={K} "
         f"slabMB={M*P/1e6:.1f}")

    # dis in new-id space (0 on pads)
    dis_new = np.zeros(NCORES * slots + 1, dtype=np.float32)
    for c in range(NCORES):
        m = nid_grid[c] >= 0
        s = np.arange(slots)[m]
        dis_new[c * slots + s] = dis[nid_grid[c][m]]

    # per-core norm grids [P, R], 0 for pads.  Layer 1 aggregates the y table
    # which already folds dis[src] (y = (dis*x) @ W1), so norm1 = dis[dst]
    # only; layer 2 aggregates plain h1@W2, so norm2 = dis[src]*dis[dst].
    tile_of_col = np.zeros(R, dtype=np.int64)
    for t in range(tiles):
        tile_of_col[offs[t]:offs[t] + D[t]] = t
    norms1, norms2, idx_safe = [], [], []
    for c in range(NCORES):
        idx = idx_new[c]                              # [P, R]
        safe = np.where(idx >= 0, idx, 0)
        dst_new = (c * slots + tile_of_col[None, :] * P
                   + np.arange(P)[:, None])           # [P, R]
        n1 = np.broadcast_to(dis_new[dst_new], idx.shape).copy()
        n1[idx < 0] = 0.0
        n2 = dis_new[safe] * dis_new[dst_new]
        n2[idx < 0] = 0.0
        norms1.append(n1.astype(np.float32))
        norms2.append(n2.astype(np.float32))
        idx_safe.append(safe)

    # weights / constants
    W1h = np.asarray(W1, np.float32)
    b1v = np.asarray(b1, np.float32)
    W2h = np.asarray(W2, np.float32)
    b2v = np.asarray(b2, np.float32)
    w3 = np.asarray(W3, np.float32)
    w3diff = (w3[:, 1] - w3[:, 0]).astype(np.float32)
    b3v = np.asarray(b3, np.float32)
    b3diff = float(b3v[1] - b3v[0])

    ident2 = np.zeros((P, 2 * P), np.float32)
    ident2[:, :P] = np.eye(P)
    ident2[:, P:] = np.eye(P)
    id2_8 = ident2.astype(ml_dtypes.float8_e4m3fn).view(np.uint8)
    w2blk = np.zeros((P, P), np.float32)
    w2blk[:HID, :HID] = W2h
    w2blk[HID:, HID:] = W2h
    w2blk = w2blk.astype(ml_dtypes.bfloat16)
    b1pair = np.concatenate([b1v, b1v]).reshape(P, 1).astype(np.float32)
    b2pair = np.concatenate([b2v, b2v]).reshape(P, 1).astype(np.float32)
    w3pair = np.zeros((P, 2), np.float32)
    w3pair[:HID, 0] = w3diff
    w3pair[HID:, 1] = w3diff
    w3pair = w3pair.astype(ml_dtypes.bfloat16)

    # ---- dispatch X: yT = W1^T @ (dis*x)T per core ----
    xT = []
    for c in range(NCORES):
        xc = np.zeros((slots, n_feat), np.float32)
        m = nid_grid[c] >= 0
        ids = nid_grid[c][m]
        xc[m] = x[ids] * dis[ids][:, None]
        xT.append(np.ascontiguousarray(xc.T).astype(np.float16))
    ncX = _build_X(slots)
    rX = _Runner(ncX, replicated=("w1d",))
    rX.stage([{"xTd": xT[c], "w1d": W1h.astype(np.float16)}
              for c in range(NCORES)])
    _log(f"staged X {time.time()-t_start:.1f}s")
    resX = rX.run()
    _log(f"ran X {time.time()-t_start:.1f}s")

    # y table in new-id space (+ trailing zero row never indexed; pads=0)
    ytab = np.zeros((NCORES * slots, HID), dtype=np.float32)
    for c in range(NCORES):
        ytab[c * slots:(c + 1) * slots] = resX[c]["yd"].astype(np.float32).T

    # ---- dispatch A ----
    msg1 = []
    for c in range(NCORES):
        msgv = ytab[idx_safe[c]] * norms1[c][:, :, None]  # [P, R, HID]
        msgv = np.concatenate(
            [msgv, np.zeros((P, 1, HID), np.float32)], axis=1)
        msg1.append(_build_slab(msgv, cols_even, cols_odd))
    _log(f"slab1 {time.time()-t_start:.1f}s")
    ncA = _build_A(groups, chunks, M, npairs)
    rA = _Runner(ncA, replicated=("id2d", "w2blkd", "b1paird"))
    rA.stage([{"msgd": msg1[c], "id2d": id2_8, "w2blkd": w2blk,
               "b1paird": b1pair} for c in range(NCORES)])
    _log(f"staged A {time.time()-t_start:.1f}s")
    resA = rA.run()
    _log(f"ran A {time.time()-t_start:.1f}s")

    # layer-2 contribution table: s2tab[newid] from shard2 outputs
    s2tab = np.zeros((NCORES * slots, HID), dtype=np.float32)
    for c in range(NCORES):
        sh = resA[c]["shard2"].astype(np.float32)     # [128, npairs*128]
        sh = sh.reshape(2, HID, npairs, P)            # [parity, f, j, p]
        s2tab[c * slots:(c + 1) * slots] = (
            sh.transpose(2, 0, 3, 1).reshape(slots, HID))
    _log(f"s2tab {time.time()-t_start:.1f}s")

    # ---- dispatch B ----
    msg2 = []
    for c in range(NCORES):
        msgv = s2tab[idx_safe[c]] * norms2[c][:, :, None]
        msgv = np.concatenate(
            [msgv, np.zeros((P, 1, HID), np.float32)], axis=1)
        msg2.append(_build_slab(msgv, cols_even, cols_odd))
    _log(f"slab2 {time.time()-t_start:.1f}s")
    ncB = _build_B(groups, chunks, M, npairs, b3diff)
    rB = _Runner(ncB, replicated=("id2d", "w3paird", "b2paird"))
    rB.stage([{"msg2d": msg2[c], "id2d": id2_8, "w3paird": w3pair,
               "b2paird": b2pair} for c in range(NCORES)])
    _log(f"staged B {time.time()-t_start:.1f}s")
    resB = rB.run()
    _log(f"ran B {time.time()-t_start:.1f}s")

    # ---- unshard: outd [128, 2*tiles] -> [n, 2] in original order ----
    full = np.empty((NCORES * slots, 2), dtype=np.float32)
    # new slot s = (2jj + par)*128 + p  <->  out[p, par*npairs + jj]
    s = np.arange(slots)
    jj, par, pp = s // (2 * P), (s // P) % 2, s % P
    q = par * npairs + jj
    for c in range(NCORES):
        o = resB[c]["out"]                            # [128, 2*tiles]
        full[c * slots:(c + 1) * slots, 0] = o[pp, q]
        full[c * slots:(c + 1) * slots, 1] = o[pp, tiles + q]
    out = full[newid]
    # keep runners alive for optional re-timing by test harness
    kernel._last = dict(rX=rX, rA=rA, rB=rB)
    kernel._dbg = dict(g=g, ytab=ytab, s2tab=s2tab, full=full, dis=dis)
    _log(f"done {time.time()-t_start:.1f}s")
    return out.astype(np.float32)


# revision 46
# speedup vs baseline: 1.3411x; 1.3411x over previous
"""Trainium2 Bass kernel for a 2-layer GCN + linear head + log_softmax
(nn_Detector_57604101373957).

Strategy (8 NeuronCores, SPMD, feature-major aggregation on the PE array):
  - Nodes are dealt to cores by global degree rank, each core's 12.5k nodes
    degree-sorted into 98 tiles of 128 slots (as in the earlier round-robin
    design).  Edges are partitioned by destination; the host pre-gathers
    per-edge messages (with the full dis[src]*dis[dst] norm folded in) into
    fp8e4m3 slabs laid out FEATURE-major: partition dim = 2x64 features of a
    tile PAIR, free dim = 128 destination slots.  One slab block per
    (group, round).
  - Aggregation = identity-matmul accumulation into PSUM on the tensor
    engine (2.4 GHz, fp8 moving operand; DoubleRow sums two rounds per
    instruction).  This keeps the DVE free and makes fp8 streaming usable
    (DVE cannot pack fp8, PE can).
  - Feature-major means zero transposes: h1T = Relu(psum + b1) directly via
    the scalar engine (bias is per-partition), layer-2 contribution
    s2T = W2_blockdiag^T @ h1T with a stationary weight, and the 2-class
    head z = w3pair^T @ h2T as a [2, N] matmul.
  - Three dispatches: X computes yT = W1^T @ (dis*x)T per core (so layer-1
    messages are 64-wide, halving slab bytes); host all-gathers y and builds
    slab1; A aggregates layer 1 + applies W2; host gathers slab2 from the
    shard outputs; B aggregates layer 2 + head + log-softmax.
"""
import os
import sys
import time

sys.path.insert(0, '/opt/trn_rl_repo')

# This kernel needs the axon-tunneled NeuronCores; undo a cpu-only pin if jax
# hasn't been initialized yet.
_jp = os.environ.get("JAX_PLATFORMS")
if _jp and "axon" not in _jp and "jax" not in sys.modules:
    os.environ.pop("JAX_PLATFORMS", None)

import numpy as np
import ml_dtypes

NCORES = 8
HID = 64
P = 128
GRP = 4           # tile-pairs per PSUM bank group (4*128 slots = 512 f32)
CHUNK_B = 4096    # slab DMA chunk size per partition (bytes)
USE_DR = bool(int(os.environ.get("GCN_USE_DOUBLEROW", "1")))
SLAB_SCALE = 32.0  # pre-quantization gain keeping fp8e4m3 in its normal range

_DEBUG = bool(int(os.environ.get("GCN_KERNEL_DEBUG", "0")))


def _log(*a):
    if _DEBUG:
        print("[kernel]", *a, flush=True)


# ----------------------------------------------------------------------------
# toolchain workarounds
# ----------------------------------------------------------------------------
_patched = False


def _apply_patches():
    """This walrus build accepts only ONE semaphore wait per instruction.
    Split Tile's tail-drain waits and any multi-wait instruction onto NOPs."""
    global _patched
    if _patched:
        return
    _patched = True
    import concourse.tile as tile_mod
    from bass_rust import ScopedClock

    def _drain_and_barrier(self, tick_clock, wait_clock):
        nc = self.nc
        import concourse.mybir as mybir
        sink = nc.sync.nop(nofuse=True)
        sink_inst = sink.ins if hasattr(sink, "ins") else sink
        wait_clock.add_sem_waits(sink_inst,
                                ScopedClock({None: tick_clock.global_clock}))
        si = sink_inst.sync_info
        waits = list(si.on_wait) if si is not None and si.on_wait else []
        if len(waits) > 1:
            si.on_wait = waits[:1]
            for k, w in enumerate(waits[1:]):
                extra = nc.sync.nop(nofuse=True)
                extra_inst = extra.ins if hasattr(extra, "ins") else extra
                esi = extra_inst.sync_info
                if esi is None:
                    extra_inst.sync_info = mybir.SyncInfo(on_wait=[w],
                                                          on_update=[])
                else:
                    esi.on_wait = [w]
        nc.sync.drain()
        nc.all_engine_barrier()
        assert self.sems is not None
        popped = nc._tile_sem_poison_stack.pop()
        assert popped is self._sem_poison
        nc.clear_and_free_semaphores(list(self.sems.allocated().values()))
        nc.all_engine_barrier()

    tile_mod.TileContext._drain_and_barrier = _drain_and_barrier


def _split_multi_waits(nc):
    import concourse.mybir as mybir
    n = 0
    for fn in nc.m.functions:
        for bb in fn.blocks:
            new_insts = []
            for inst in bb.instructions:
                si = inst.sync_info
                if si is not None and si.on_wait and len(si.on_wait) > 1:
                    waits = list(si.on_wait)
                    for k, w in enumerate(waits[:-1]):
                        nop = mybir.InstNoOp(
                            name=f"{inst.name}-wsplit{k}",
                            sync_info=mybir.SyncInfo(on_wait=[w],
                                                     on_update=[]),
                            bass_nofuse=True,
                            engine=inst.engine,
                        )
                        new_insts.append(nop)
                    si.on_wait = waits[-1:]
                    n += 1
                new_insts.append(inst)
            bb.instructions[:] = new_insts
    return n


# ----------------------------------------------------------------------------
# SPMD runner (compile once, run; mirrors bass2jax.run_bass_via_pjrt)
# ----------------------------------------------------------------------------
class _Runner:
    def __init__(self, nc, n_cores=NCORES, replicated=()):
        import jax
        from jax.sharding import Mesh, PartitionSpec
        from jax.experimental.shard_map import shard_map
        import concourse.mybir as mybir
        from concourse.bass2jax import (_bass_exec_p, install_neuronx_cc_hook,
                                        partition_id_tensor)
        install_neuronx_cc_hook()
        _split_multi_waits(nc)
        self.jax = jax
        self.n_cores = n_cores
        in_names, out_names, out_avals, zero_outs = [], [], [], []
        pname = nc.partition_id_tensor.name if nc.partition_id_tensor else None
        for alloc in nc.m.functions[0].allocations:
            if not isinstance(alloc, mybir.MemoryLocationSet):
                continue
            name = alloc.memorylocations[0].name
            if alloc.kind == "ExternalInput":
                if name != pname:
                    in_names.append(name)
            elif alloc.kind == "ExternalOutput":
                out_names.append(name)
                shape = tuple(alloc.tensor_shape)
                dtype = mybir.dt.np(alloc.dtype)
                out_avals.append(jax.core.ShapedArray(shape, dtype))
                zero_outs.append(np.zeros(shape, dtype))
        self.in_names, self.out_names = in_names, out_names
        self.out_avals, self.zero_outs = out_avals, zero_outs
        all_in = in_names + out_names + ([pname] if pname else [])

        def _body(*args):
            operands = list(args)
            if pname is not None:
                operands.append(partition_id_tensor())
            return tuple(_bass_exec_p.bind(
                *operands,
                out_avals=tuple(out_avals),
                in_names=tuple(all_in),
                out_names=tuple(out_names),
                lowering_input_output_aliases=(),
                sim_require_finite=True,
                sim_require_nnan=True,
                nc=nc,
            ))

        devices = jax.devices()[:n_cores]
        self.mesh = Mesh(np.asarray(devices), ("core",))
        self.replicated = set(replicated)
        in_specs = tuple(
            PartitionSpec() if name in self.replicated else PartitionSpec("core")
            for name in in_names) + (PartitionSpec("core"),) * len(out_names)
        out_specs = (PartitionSpec("core"),) * len(out_names)
        self.fn = jax.jit(shard_map(_body, mesh=self.mesh, in_specs=in_specs,
                                    out_specs=out_specs, check_rep=False),
                          keep_unused=True)
        self._staged = None

    def stage(self, in_maps):
        from jax.sharding import NamedSharding, PartitionSpec
        n = self.n_cores
        sh = NamedSharding(self.mesh, PartitionSpec("core"))
        shr = NamedSharding(self.mesh, PartitionSpec())
        staged = []
        for name in self.in_names:
            if name in self.replicated:
                staged.append(self.jax.device_put(
                    np.asarray(in_maps[0][name]), shr))
            else:
                staged.append(self.jax.device_put(np.concatenate(
                    [np.asarray(in_maps[c][name]) for c in range(n)], axis=0),
                    sh))
        staged += [self.jax.device_put(
            np.zeros((n * z.shape[0], *z.shape[1:]), z.dtype), sh)
            for z in self.zero_outs]
        self._staged = staged
        self.jax.block_until_ready(self._staged)

    def run(self):
        out = self.fn(*self._staged)
        self.jax.block_until_ready(out)
        n = self.n_cores
        out = [np.asarray(o) for o in out]
        return [{name: out[i].reshape(n, *self.out_avals[i].shape)[c]
                 for i, name in enumerate(self.out_names)}
                for c in range(n)]

    def time_once(self):
        t0 = time.perf_counter()
        out = self.fn(*self._staged)
        self.jax.block_until_ready(out)
        return time.perf_counter() - t0

    def time_pipelined(self, n_iter=10, warmup=2):
        for _ in range(warmup):
            out = self.fn(*self._staged)
        self.jax.block_until_ready(out)
        t0 = time.perf_counter()
        outs = [self.fn(*self._staged) for _ in range(n_iter)]
        self.jax.block_until_ready(outs)
        return (time.perf_counter() - t0) / n_iter


# ----------------------------------------------------------------------------
# host-side graph preparation
# ----------------------------------------------------------------------------
def _prep_graph(edge_index, n):
    src = np.asarray(edge_index[0], dtype=np.int64)
    dst = np.asarray(edge_index[1], dtype=np.int64)
    deg = np.bincount(dst, minlength=n).astype(np.int64) + 1  # + self loop

    # deal nodes to cores by global degree rank
    order = np.argsort(-deg, kind="stable")          # rank -> orig node
    rank_of = np.empty(n, dtype=np.int64)
    rank_of[order] = np.arange(n)
    core_of = rank_of % NCORES
    slot_of = rank_of // NCORES                      # degree-sorted per core

    per_core = (n + NCORES - 1) // NCORES
    tiles = (per_core + P - 1) // P
    slots = tiles * P
    newid = core_of * slots + slot_of                # orig -> new id

    # common per-tile round counts: max degree of slot t*128 across cores
    # (slots are degree-sorted descending, so tile max = first slot's degree)
    D = np.zeros(tiles, dtype=np.int64)
    deg_sorted = deg[order]                          # descending
    for t in range(tiles):
        s0 = t * P
        ranks = s0 * NCORES + np.arange(NCORES)      # first slot of tile t
        ranks = ranks[ranks < n]
        D[t] = deg_sorted[ranks].max() if len(ranks) else 1
    R = int(D.sum())
    offs = np.zeros(tiles, dtype=np.int64)
    offs[1:] = np.cumsum(D)[:-1]

    # in-edge lists grouped by destination (new-id space), self-loops first
    e_order = np.argsort(newid[dst], kind="stable")
    sdst_new = newid[dst][e_order]
    ssrc_new = newid[src][e_order]
    starts = np.searchsorted(sdst_new, np.arange(NCORES * slots), side="left")
    ends = np.searchsorted(sdst_new, np.arange(NCORES * slots), side="right")

    # per-core gather index arrays [128, R] (new-id space), -1 for pad
    nid_grid = np.full((NCORES, slots), -1, dtype=np.int64)
    nid_grid[core_of[order], slot_of[order]] = order  # orig ids on the grid
    idx_new = np.full((NCORES, P, R), -1, dtype=np.int64)
    for t in range(tiles):
        dt = int(D[t])
        o = int(offs[t])
        for c in range(NCORES):
            base = c * slots + t * P
            for p in range(P):
                v = nid_grid[c, t * P + p]
                if v < 0:
                    continue
                nv = base + p
                s, e = starts[nv], ends[nv]
                cnt = e - s
                # self-loop first, then in-edges
                idx_new[c, p, o] = nv
                m = min(cnt, dt - 1)
                idx_new[c, p, o + 1:o + 1 + m] = ssrc_new[s:s + m]
                assert cnt <= dt - 1, (cnt, dt)
    return dict(order=order, newid=newid, core_of=core_of, slot_of=slot_of,
                deg=deg, tiles=tiles, slots=slots, D=D, R=R, offs=offs,
                idx_new=idx_new, nid_grid=nid_grid)


def _pair_groups(tiles, D):
    """Uniform-width group/round structure shared by host slab layout and the
    device programs.  Returns (npairs, groups, chunks) where groups is a list
    of (pair0, wg, Rg) and chunks[g] is a list of round counts per DMA."""
    assert tiles % 2 == 0
    npairs = tiles // 2
    Dp = [int(D[2 * j]) for j in range(npairs)]      # per-pair rounds
    # DP over group sizes 1..GRP minimizing padded slab columns sum(wg*Rg)
    # plus instruction overheads: each round-matmul ~1.5 column-equivalents
    # of issue cost, each group ~12 (epilogue ops + weight swap).
    INF = 1 << 60
    OH_ROUND, OH_GROUP = 3, 24                       # in half-columns
    best = [INF] * (npairs + 1)
    bsz = [0] * (npairs + 1)
    best[npairs] = 0
    for j in range(npairs - 1, -1, -1):
        for s in range(1, min(GRP, npairs - j) + 1):
            c = 2 * s * Dp[j] + OH_ROUND * Dp[j] + OH_GROUP + best[j + s]
            if c < best[j]:
                best[j], bsz[j] = c, s
    groups = []
    j = 0
    while j < npairs:
        wg = bsz[j]
        groups.append((j, wg, Dp[j]))                # D sorted desc -> max
        j += wg
    chunks = []
    for (j0, wg, Rg) in groups:
        rpc = max(1, CHUNK_B // (wg * P))            # rounds per chunk
        ch = []
        r = 0
        while r < Rg:
            ch.append(min(rpc, Rg - r))
            r += rpc
        chunks.append(ch)
    return npairs, Dp, groups, chunks


def _slab_cols(groups, Dp, D, offs):
    """Column index arrays (into the per-core [P, R] message grid, R = pad)
    in exact device-emission order.  Returns (cols_even, cols_odd, ncolblk)."""
    ce, co = [], []
    R = int(D.sum())
    for (j0, wg, Rg) in groups:
        for r in range(Rg):
            for jj in range(wg):
                j = j0 + jj
                te, to = 2 * j, 2 * j + 1
                ce.append(offs[te] + r if r < D[te] else R)
                co.append(offs[to] + r if r < D[to] else R)
    return np.asarray(ce, dtype=np.int64), np.asarray(co, dtype=np.int64)


def _build_slab(msgv, cols_even, cols_odd):
    """msgv: [P(slot), R+1, HID] f32 message grid (last col all-zero).
    Returns the fp8 feature-major slab bytes [128, K*128] uint8."""
    K = len(cols_even)
    top = msgv[:, cols_even, :]                      # [128, K, 64]
    bot = msgv[:, cols_odd, :]
    # -> [64 feats, K, 128 slots] stacked to [128, K*128]
    slab = np.empty((2 * HID, K, P), dtype=np.float32)
    slab[:HID] = top.transpose(2, 1, 0)
    slab[HID:] = bot.transpose(2, 1, 0)
    slab *= SLAB_SCALE
    # TRN fp8e4 treats exp=1111 as inf/nan (unlike OCP e4m3fn): clip to 240
    np.clip(slab, -240.0, 240.0, out=slab)
    slab8 = slab.reshape(2 * HID, K * P).astype(ml_dtypes.float8_e4m3fn)
    return slab8.view(np.uint8)


# ----------------------------------------------------------------------------
# bass programs
# ----------------------------------------------------------------------------
def _emit_agg(nc, mpool, pspool, msgd, id2f8, groups, chunks, mcol_state):
    """Emit the aggregation matmul stream for one group list; returns a list
    of per-group PSUM tiles (caller emits epilogues with 1-group lookahead).
    Chunk DMAs alternate between the two HWDGE queues (SP / Activation) so
    transfers don't serialize on one queue engine."""
    import concourse.mybir as mybir
    DR = mybir.MatmulPerfMode.DoubleRow
    f32 = mybir.dt.float32
    id3 = id2f8.rearrange("k (s m) -> k s m", s=2)
    queues = [nc.sync, nc.scalar, nc.gpsimd]
    qi = [0]

    def agg_group(g):
        (j0, wg, Rg) = groups[g]
        wcols = wg * P
        ps = pspool.tile([P, GRP * P], f32, tag="agg")
        r = 0
        for nrounds in chunks[g]:
            cb = nrounds * wcols
            chunk = mpool.tile([P, CHUNK_B], mybir.dt.uint8, tag="chunk")
            queues[qi[0] % len(queues)].dma_start(
                out=chunk[:, :cb],
                in_=msgd[:, mcol_state[0]:mcol_state[0] + cb])
            qi[0] += 1
            mcol_state[0] += cb
            cf8 = chunk[:].bitcast(mybir.dt.float8e4)
            roff = 0
            rleft = nrounds
            while rleft:
                if USE_DR and rleft >= 2:
                    rhs = cf8[:, roff:roff + 2 * wcols].rearrange(
                        "k (s n) -> k s n", s=2)
                    nc.tensor.matmul(out=ps[:, :wcols], lhsT=id3, rhs=rhs,
                                     start=(r == 0), stop=(r + 2 == Rg),
                                     perf_mode=DR)
                    r += 2
                    roff += 2 * wcols
                    rleft -= 2
                else:
                    nc.tensor.matmul(out=ps[:, :wcols], lhsT=id2f8[:, :P],
                                     rhs=cf8[:, roff:roff + wcols],
                                     start=(r == 0), stop=(r + 1 == Rg))
                    r += 1
                    roff += wcols
                    rleft -= 1
        return ps

    return agg_group


def _build_A(groups, chunks, M, npairs, b_zero=True):
    """Layer-1 aggregation (of host-gathered y messages) + W2 transform.
    Output shard2 f16 [128, npairs*128]: partition q<64 = feat q of even
    tile, q>=64 = feat q-64 of odd tile; col = pair*128 + slot."""
    import concourse.bass as bass
    import concourse.mybir as mybir
    import concourse.tile as tile
    f32, f16, bf16 = mybir.dt.float32, mybir.dt.float16, mybir.dt.bfloat16
    u8 = mybir.dt.uint8

    nc = bass.Bass()
    msgd = nc.dram_tensor("msgd", [P, M], u8, kind="ExternalInput")
    id2d = nc.dram_tensor("id2d", [P, 2 * P], u8, kind="ExternalInput")
    w2blkd = nc.dram_tensor("w2blkd", [P, P], bf16, kind="ExternalInput")
    b1paird = nc.dram_tensor("b1paird", [P, 1], f32, kind="ExternalInput")
    shard2 = nc.dram_tensor("shard2", [P, npairs * P], f16,
                            kind="ExternalOutput")

    with tile.TileContext(nc) as tc:
        with tc.tile_pool(name="const", bufs=1) as cpool, \
             tc.tile_pool(name="msg", bufs=6) as mpool, \
             tc.tile_pool(name="ep", bufs=2) as epool, \
             tc.tile_pool(name="ps", bufs=2, space="PSUM") as pspool:
            id2 = cpool.tile([P, 2 * P], u8)
            nc.sync.dma_start(out=id2[:], in_=id2d[:, :])
            w2blk = cpool.tile([P, P], bf16)
            nc.sync.dma_start(out=w2blk[:], in_=w2blkd[:, :])
            b1p = cpool.tile([P, 1], f32)
            nc.sync.dma_start(out=b1p[:], in_=b1paird[:, :])
            id2f8 = id2[:].bitcast(mybir.dt.float8e4)

            mcol = [0]
            agg_group = _emit_agg(nc, mpool, pspool, msgd, id2f8, groups,
                                  chunks, mcol)
            dqs = [nc.sync, nc.scalar, nc.gpsimd]

            def epilogue(g, ps):
                (j0, wg, Rg) = groups[g]
                wcols = wg * P
                # h1' = relu(psum [+ S*b1]); the 1/SLAB_SCALE rescale is
                # folded into w2blk on the host (relu(S x) = S relu(x))
                h1 = epool.tile([P, GRP * P], bf16, tag="h1")
                if b_zero:
                    nc.vector.tensor_scalar_max(out=h1[:, :wcols],
                                                in0=ps[:, :wcols],
                                                scalar1=0.0)
                else:
                    nc.scalar.activation(
                        out=h1[:, :wcols], in_=ps[:, :wcols],
                        func=mybir.ActivationFunctionType.Relu,
                        bias=b1p[:], scale=1.0)
                ps2 = pspool.tile([P, GRP * P], f32, tag="ps2")
                nc.tensor.matmul(out=ps2[:, :wcols], lhsT=w2blk[:],
                                 rhs=h1[:, :wcols], start=True, stop=True)
                s2 = epool.tile([P, GRP * P], f16, tag="s2")
                nc.vector.tensor_copy(out=s2[:, :wcols], in_=ps2[:, :wcols])
                dqs[g % 3].dma_start(
                    out=shard2[:, j0 * P:j0 * P + wcols], in_=s2[:, :wcols])

            prev = None
            for g in range(len(groups)):
                ps = agg_group(g)
                if prev is not None:
                    epilogue(g - 1, prev)
                prev = ps
            epilogue(len(groups) - 1, prev)
            assert mcol[0] == M, (mcol[0], M)
    return nc


def _build_B(groups, chunks, M, npairs, b3diff, b_zero=True):
    """Layer-2 aggregation + relu + 2-class head + log_softmax.
    z' rows [2, N] are bounced through a DRAM scratch tile into a dense
    [128, 98] layout so the head math runs on full partitions.
    Output outd f32 [128, 2*tiles]: out[p, 2*(jj*2+c)/..]: col q<tiles is
    lp0 for new slot (q)*128+p remapped as q=jj*2+c; col tiles+q is lp1."""
    import concourse.bass as bass
    import concourse.mybir as mybir
    import concourse.tile as tile
    f32, f16, bf16 = mybir.dt.float32, mybir.dt.float16, mybir.dt.bfloat16
    u8 = mybir.dt.uint8
    N = npairs * P
    tiles = 2 * npairs

    nc = bass.Bass()
    msgd = nc.dram_tensor("msg2d", [P, M], u8, kind="ExternalInput")
    id2d = nc.dram_tensor("id2d", [P, 2 * P], u8, kind="ExternalInput")
    w3paird = nc.dram_tensor("w3paird", [P, 2], bf16, kind="ExternalInput")
    b2paird = nc.dram_tensor("b2paird", [P, 1], f32, kind="ExternalInput")
    outd = nc.dram_tensor("out", [P, 2 * tiles], f32, kind="ExternalOutput")

    with tile.TileContext(nc) as tc:
        with tc.tile_pool(name="const", bufs=1) as cpool, \
             tc.tile_pool(name="msg", bufs=6) as mpool, \
             tc.tile_pool(name="ep", bufs=2) as epool, \
             tc.tile_pool(name="z", bufs=1) as zpool, \
             tc.tile_pool(name="zd", bufs=1, space="DRAM") as zdpool, \
             tc.tile_pool(name="ps", bufs=2, space="PSUM") as pspool:
            id2 = cpool.tile([P, 2 * P], u8)
            nc.sync.dma_start(out=id2[:], in_=id2d[:, :])
            w3p = cpool.tile([P, 2], bf16)
            nc.sync.dma_start(out=w3p[:], in_=w3paird[:, :])
            b2p = cpool.tile([P, 1], f32)
            nc.sync.dma_start(out=b2p[:], in_=b2paird[:, :])
            id2f8 = id2[:].bitcast(mybir.dt.float8e4)

            # warm the Relu/Exp/Ln act-table while the stream runs (the head
            # would otherwise pay the ~1.3us table load in its serial tail)
            warm = cpool.tile([P, 1], f32)
            nc.scalar.activation(out=warm[:], in_=b2p[:],
                                 func=mybir.ActivationFunctionType.Relu)

            # per-group z' slices bounce through DRAM [2*ngroups, GRP*128] so
            # the head can run on a dense [128, tiles] tile (dim0=128 DMAs)
            zdram = zdpool.tile([2 * len(groups), GRP * P], f32)

            mcol = [0]
            agg_group = _emit_agg(nc, mpool, pspool, msgd, id2f8, groups,
                                  chunks, mcol)
            dqs = [nc.sync, nc.scalar, nc.gpsimd]

            def epilogue(g, ps):
                (j0, wg, Rg) = groups[g]
                wcols = wg * P
                # h2' = relu(psum [+ S*b2]); 1/SLAB_SCALE folded into w3pair
                h2 = epool.tile([P, GRP * P], bf16, tag="h2")
                if b_zero:
                    nc.vector.tensor_scalar_max(out=h2[:, :wcols],
                                                in0=ps[:, :wcols],
                                                scalar1=0.0)
                else:
                    nc.scalar.activation(
                        out=h2[:, :wcols], in_=ps[:, :wcols],
                        func=mybir.ActivationFunctionType.Relu,
                        bias=b2p[:], scale=1.0)
                psz = pspool.tile([2, GRP * P], f32, tag="psz")
                nc.tensor.matmul(out=psz[:, :wcols], lhsT=w3p[:],
                                 rhs=h2[:, :wcols], start=True, stop=True)
                # b3diff is applied in the head; stage z and dump to DRAM
                zs = epool.tile([2, GRP * P], f32, tag="zs")
                nc.vector.tensor_copy(out=zs[:, :wcols], in_=psz[:, :wcols])
                dqs[g % 3].dma_start(out=zdram[2 * g:2 * g + 2, :wcols],
                                     in_=zs[:, :wcols])

            prev = None
            for g in range(len(groups)):
                ps = agg_group(g)
                if prev is not None:
                    epilogue(g - 1, prev)
                prev = ps
            epilogue(len(groups) - 1, prev)
            assert mcol[0] == M, (mcol[0], M)

            # gather z' into dense [128, tiles]: col q = group-blocked
            # (c, jj); 13 tiny strided reads (32B/partition each)
            zd = zpool.tile([P, tiles], f32)
            qb = 0
            for g, (j0, wg, Rg) in enumerate(groups):
                for c2 in range(2):
                    dqs[(2 * g + c2) % 3].dma_start(
                        out=zd[:, qb + c2 * wg:qb + (c2 + 1) * wg],
                        in_=zdram[2 * g + c2:2 * g + c2 + 1,
                                  :wg * P].rearrange("c (jj p) -> (c p) jj",
                                                     p=P))
                qb += 2 * wg
            assert qb == tiles

            # head: z' = z + b3diff; lp0 = -sp(z'), lp1 = z' - sp(z');
            # stable sp(z) = m + ln(1 + exp(z - 2m)), m = relu(z)
            Act = mybir.ActivationFunctionType
            zp = zpool.tile([P, tiles], f32)
            nc.vector.tensor_scalar_add(out=zp[:], in0=zd[:],
                                        scalar1=float(b3diff))
            zd = zp
            m = zpool.tile([P, tiles], f32)
            nc.scalar.activation(out=m[:], in_=zd[:], func=Act.Relu)
            e = zpool.tile([P, tiles], f32)
            nc.vector.tensor_scalar(out=e[:], in0=m[:], scalar1=-2.0,
                                    scalar2=None, op0=mybir.AluOpType.mult)
            nc.vector.tensor_tensor(out=e[:], in0=e[:], in1=zd[:],
                                    op=mybir.AluOpType.add)
            nc.scalar.activation(out=e[:], in_=e[:], func=Act.Exp)
            # u = ln(e + 1)
            nc.scalar.activation(out=e[:], in_=e[:], func=Act.Ln, bias=1.0)
            sp = zpool.tile([P, tiles], f32)
            nc.vector.tensor_tensor(out=sp[:], in0=m[:], in1=e[:],
                                    op=mybir.AluOpType.add)
            lp = zpool.tile([P, 2 * tiles], f32)
            nc.vector.tensor_scalar_mul(out=lp[:, :tiles], in0=sp[:],
                                        scalar1=-1.0)
            nc.vector.tensor_tensor(out=lp[:, tiles:], in0=zd[:], in1=sp[:],
                                    op=mybir.AluOpType.subtract)
            nc.sync.dma_start(out=outd[:, :], in_=lp[:])
    return nc


def _build_X(slots):
    """yT = (W1/S)^T @ xT per core: xT fp8 [128 feats, slots] (dis-scaled x
    pre-scaled by S on host), output yd f16 [64, slots] (true scale)."""
    import concourse.bass as bass
    import concourse.mybir as mybir
    import concourse.tile as tile
    f32, f16, u8 = mybir.dt.float32, mybir.dt.float16, mybir.dt.uint8

    nc = bass.Bass()
    xTd = nc.dram_tensor("xTd", [P, slots], u8, kind="ExternalInput")
    w1d = nc.dram_tensor("w1d", [P, HID], f16, kind="ExternalInput")
    yd = nc.dram_tensor("yd", [HID, slots], f16, kind="ExternalOutput")

    with tile.TileContext(nc) as tc:
        with tc.tile_pool(name="const", bufs=1) as cpool, \
             tc.tile_pool(name="st", bufs=4) as spool, \
             tc.tile_pool(name="ps", bufs=3, space="PSUM") as pspool:
            w1 = cpool.tile([P, HID], f16)
            nc.sync.dma_start(out=w1[:], in_=w1d[:, :])
            CN = 1024
            nch = (slots + CN - 1) // CN
            queues = [nc.gpsimd, nc.scalar, nc.sync]
            for k in range(nch):
                c0 = k * CN
                w = min(CN, slots - c0)
                xch = spool.tile([P, CN], u8, tag="xch")
                queues[k % len(queues)].dma_start(out=xch[:, :w],
                                                  in_=xTd[:, c0:c0 + w])
                xf8 = xch[:].bitcast(mybir.dt.float8e4)
                psy = pspool.tile([HID, CN], f32, tag="psy")
                for o in range(0, w, 512):
                    wo = min(512, w - o)
                    nc.tensor.matmul(out=psy[:, o:o + wo], lhsT=w1[:],
                                     rhs=xf8[:, o:o + wo],
                                     start=True, stop=True)
                yst = spool.tile([HID, CN], f16, tag="yst")
                if k % 2 == 0:
                    nc.vector.tensor_copy(out=yst[:, :w], in_=psy[:, :w])
                else:
                    nc.scalar.activation(
                        out=yst[:, :w], in_=psy[:, :w],
                        func=mybir.ActivationFunctionType.Copy)
                queues[(k + 1) % len(queues)].dma_start(
                    out=yd[:, c0:c0 + w], in_=yst[:, :w])
    return nc


# ----------------------------------------------------------------------------
# main entry
# ----------------------------------------------------------------------------
def kernel(x, edge_index, W1, b1, W2, b2, W3, b3):
    _apply_patches()
    x = np.asarray(x, dtype=np.float32)
    n, n_feat = x.shape
    t_start = time.time()
    g = _prep_graph(edge_index, n)
    tiles, slots, D, R = g["tiles"], g["slots"], g["D"], g["R"]
    offs, idx_new, nid_grid = g["offs"], g["idx_new"], g["nid_grid"]
    newid = g["newid"]
    deg = g["deg"]
    dis = (1.0 / np.sqrt(deg)).astype(np.float32)

    npairs, Dp, groups, chunks = _pair_groups(tiles, D)
    cols_even, cols_odd = _slab_cols(groups, Dp, D, offs)
    K = len(cols_even)
    M = K * P
    _log(f"prep {time.time()-t_start:.1f}s tiles={tiles} R={R} K={K} "
         f"slabMB={M*P/1e6:.1f}")

    # dis in new-id space (0 on pads)
    dis_new = np.zeros(NCORES * slots + 1, dtype=np.float32)
    for c in range(NCORES):
        m = nid_grid[c] >= 0
        s = np.arange(slots)[m]
        dis_new[c * slots + s] = dis[nid_grid[c][m]]

    # per-core norm grids [P, R], 0 for pads.  Layer 1 aggregates the y table
    # which already folds dis[src] (y = (dis*x) @ W1), so norm1 = dis[dst]
    # only; layer 2 aggregates plain h1@W2, so norm2 = dis[src]*dis[dst].
    tile_of_col = np.zeros(R, dtype=np.int64)
    for t in range(tiles):
        tile_of_col[offs[t]:offs[t] + D[t]] = t
    norms1, norms2, idx_safe = [], [], []
    for c in range(NCORES):
        idx = idx_new[c]                              # [P, R]
        safe = np.where(idx >= 0, idx, 0)
        dst_new = (c * slots + tile_of_col[None, :] * P
                   + np.arange(P)[:, None])           # [P, R]
        n1 = np.broadcast_to(dis_new[dst_new], idx.shape).copy()
        n1[idx < 0] = 0.0
        n2 = dis_new[safe] * dis_new[dst_new]
        n2[idx < 0] = 0.0
        norms1.append(n1.astype(np.float32))
        norms2.append(n2.astype(np.float32))
        idx_safe.append(safe)

    # weights / constants
    W1h = np.asarray(W1, np.float32)
    b1v = np.asarray(b1, np.float32)
    W2h = np.asarray(W2, np.float32)
    b2v = np.asarray(b2, np.float32)
    w3 = np.asarray(W3, np.float32)
    w3diff = (w3[:, 1] - w3[:, 0]).astype(np.float32)
    b3v = np.asarray(b3, np.float32)
    b3diff = float(b3v[1] - b3v[0])

    ident2 = np.zeros((P, 2 * P), np.float32)
    ident2[:, :P] = np.eye(P)
    ident2[:, P:] = np.eye(P)
    id2_8 = ident2.astype(ml_dtypes.float8_e4m3fn).view(np.uint8)
    # h' = relu(psum) keeps the SLAB_SCALE gain; fold 1/S into the next
    # weight (and S into the bias for the non-zero-bias path)
    b1zero = not np.any(b1v)
    b2zero = not np.any(b2v)
    w2blk = np.zeros((P, P), np.float32)
    w2blk[:HID, :HID] = W2h
    w2blk[HID:, HID:] = W2h
    w2blk = (w2blk / SLAB_SCALE).astype(ml_dtypes.bfloat16)
    b1pair = (np.concatenate([b1v, b1v]).reshape(P, 1)
              * SLAB_SCALE).astype(np.float32)
    b2pair = (np.concatenate([b2v, b2v]).reshape(P, 1)
              * SLAB_SCALE).astype(np.float32)
    w3pair = np.zeros((P, 2), np.float32)
    w3pair[:HID, 0] = w3diff
    w3pair[HID:, 1] = w3diff
    w3pair = (w3pair / SLAB_SCALE).astype(ml_dtypes.bfloat16)

    # ---- dispatch X: yT = W1^T @ (dis*x)T per core (fp8 input, gain XS
    # folded into the f16 weights) ----
    XS = 32.0
    xT = []
    for c in range(NCORES):
        xc = np.zeros((slots, n_feat), np.float32)
        m = nid_grid[c] >= 0
        ids = nid_grid[c][m]
        xc[m] = x[ids] * dis[ids][:, None]
        xs = np.clip(np.ascontiguousarray(xc.T) * XS, -240.0, 240.0)
        xT.append(xs.astype(ml_dtypes.float8_e4m3fn).view(np.uint8))
    ncX = _build_X(slots)
    rX = _Runner(ncX, replicated=("w1d",))
    rX.stage([{"xTd": xT[c], "w1d": (W1h / XS).astype(np.float16)}
              for c in range(NCORES)])
    _log(f"staged X {time.time()-t_start:.1f}s")
    resX = rX.run()
    _log(f"ran X {time.time()-t_start:.1f}s")

    # y table in new-id space (+ trailing zero row never indexed; pads=0)
    ytab = np.zeros((NCORES * slots, HID), dtype=np.float32)
    for c in range(NCORES):
        ytab[c * slots:(c + 1) * slots] = resX[c]["yd"].astype(np.float32).T

    # ---- dispatch A ----
    msg1 = []
    for c in range(NCORES):
        msgv = ytab[idx_safe[c]] * norms1[c][:, :, None]  # [P, R, HID]
        msgv = np.concatenate(
            [msgv, np.zeros((P, 1, HID), np.float32)], axis=1)
        msg1.append(_build_slab(msgv, cols_even, cols_odd))
    _log(f"slab1 {time.time()-t_start:.1f}s")
    ncA = _build_A(groups, chunks, M, npairs, b_zero=b1zero)
    rA = _Runner(ncA, replicated=("id2d", "w2blkd", "b1paird"))
    rA.stage([{"msgd": msg1[c], "id2d": id2_8, "w2blkd": w2blk,
               "b1paird": b1pair} for c in range(NCORES)])
    _log(f"staged A {time.time()-t_start:.1f}s")
    resA = rA.run()
    _log(f"ran A {time.time()-t_start:.1f}s")

    # layer-2 contribution table: s2tab[newid] from shard2 outputs
    s2tab = np.zeros((NCORES * slots, HID), dtype=np.float32)
    for c in range(NCORES):
        sh = resA[c]["shard2"].astype(np.float32)     # [128, npairs*128]
        sh = sh.reshape(2, HID, npairs, P)            # [parity, f, j, p]
        s2tab[c * slots:(c + 1) * slots] = (
            sh.transpose(2, 0, 3, 1).reshape(slots, HID))
    _log(f"s2tab {time.time()-t_start:.1f}s")

    # ---- dispatch B ----
    msg2 = []
    for c in range(NCORES):
        msgv = s2tab[idx_safe[c]] * norms2[c][:, :, None]
        msgv = np.concatenate(
            [msgv, np.zeros((P, 1, HID), np.float32)], axis=1)
        msg2.append(_build_slab(msgv, cols_even, cols_odd))
    _log(f"slab2 {time.time()-t_start:.1f}s")
    ncB = _build_B(groups, chunks, M, npairs, b3diff, b_zero=b2zero)
    rB = _Runner(ncB, replicated=("id2d", "w3paird", "b2paird"))
    rB.stage([{"msg2d": msg2[c], "id2d": id2_8, "w3paird": w3pair,
               "b2paird": b2pair} for c in range(NCORES)])
    _log(f"staged B {time.time()-t_start:.1f}s")
    resB = rB.run()
    _log(f"ran B {time.time()-t_start:.1f}s")

    # ---- unshard: outd [128, 2*tiles] -> [n, 2] in original order ----
    full = np.empty((NCORES * slots, 2), dtype=np.float32)
    # new slot s = (2jj + par)*128 + p <-> out[p, qb(g) + par*wg + (jj - j0)]
    q_of = np.zeros((2, npairs), dtype=np.int64)
    qb = 0
    for (j0, wg, Rg) in groups:
        for c2 in range(2):
            q_of[c2, j0:j0 + wg] = qb + c2 * wg + np.arange(wg)
        qb += 2 * wg
    s = np.arange(slots)
    jj, par, pp = s // (2 * P), (s // P) % 2, s % P
    q = q_of[par, jj]
    for c in range(NCORES):
        o = resB[c]["out"]                            # [128, 2*tiles]
        full[c * slots:(c + 1) * slots, 0] = o[pp, q]
        full[c * slots:(c + 1) * slots, 1] = o[pp, tiles + q]
    out = full[newid]
    # keep runners alive for optional re-timing by test harness
    kernel._last = dict(rX=rX, rA=rA, rB=rB)
    kernel._dbg = dict(g=g, ytab=ytab, s2tab=s2tab, full=full, dis=dis)
    _log(f"done {time.time()-t_start:.1f}s")
    return out.astype(np.float32)
